# revision 1
# baseline (speedup 1.0000x reference)
"""DigitCaps routing kernel for 8 Trainium2 NeuronCores.

Math (reference, with b-logits starting at 0):
  u_hat[b,i,c,o] = sum_v W[i,c,o,v] * x[b,i,v]
  O_t = sum_{tau<t} out_tau  (accumulated squashed outputs)  =>  logits = u_hat * O_t
  iter t: e = exp(u_hat*O); denom = sum_c e; s = sum_i (e/denom)*u_hat
  Key identity used: s = invO * sum_i (e/denom) * l  where l = u_hat*O,
  invO = O/(O^2+eps2) -- avoids re-reading u_hat (l is already in SBUF for exp).

Sharding: i (2048) split across 8 cores (256 each). W/x sliced per core on host.
Only cross-core traffic: 3 AllReduces of s [1024, 64] fp32 (256KB).

Per-core layouts (host-prepared):
  w  [128, 16*1024] fp32: w[64j+8r+v, (g, c*16+o)] = W[i0+16g+8j+r, c, o, v]
  xd [128, 16*64]  fp32: xd[64j+8r+v, (g, b)] = x[b, i0+16g+8j+r, v]
  xz [128, 8*16*64] fp32: xz[p, (r', g, b)] = xd[p, (g,b)] if r'==(p%64)//8 else 0
     (zero-padded variants so per-i K=64 matmuls pick out one i via rhs zeros;
      LDW partition base must be in {0, 32, 64})
"""
import sys

sys.path.insert(0, "/opt/trn_rl_repo")

import numpy as np

_CACHE = {}

BS, NI, NC_, OL, NV = 64, 2048, 64, 16, 8
CO = NC_ * OL          # 1024
NCORES = 8
ILOC = NI // NCORES    # 256
NG = 16                # i-groups per core
NB_G = 8               # samples per sample-group
EPS = 1e-9
EPS2 = 1e-30
TH = 1e-4
SCALE = 1024.0


def _build_program(ncores=NCORES):
    import concourse.bass as bass
    import concourse.bacc as bacc
    import concourse.mybir as mybir
    import concourse.tile as tile

    F32 = mybir.dt.float32
    BF16 = mybir.dt.bfloat16
    FP16 = mybir.dt.float16
    AO = mybir.AluOpType

    nc = bacc.Bacc("TRN2", target_bir_lowering=False, debug=False,
                   num_devices=ncores)

    w_in = nc.dram_tensor("w", [128, NG * CO], F32, kind="ExternalInput")
    xd_in = nc.dram_tensor("xd", [128, NG * BS], F32, kind="ExternalInput")
    xz_in = nc.dram_tensor("xz", [128, 8 * NG * BS], F32, kind="ExternalInput")
    ones_in = nc.dram_tensor("ones", [128, OL], FP16, kind="ExternalInput")
    out_d = nc.dram_tensor("out", [BS, NC_, OL], F32, kind="ExternalOutput")

    with tile.TileContext(nc) as tc:
        with (
            tc.tile_pool(name="big", bufs=1) as big,
            tc.tile_pool(name="lpool", bufs=8) as lpool,
            tc.tile_pool(name="epool", bufs=8) as epool,
            tc.tile_pool(name="small", bufs=1) as small,
            tc.tile_pool(name="scr", bufs=4) as scr,
            tc.tile_pool(name="psu_p", bufs=2, space="PSUM") as psu_p,
            tc.tile_pool(name="psd_p", bufs=1, space="PSUM") as psd_p,
            tc.tile_pool(name="dram", bufs=2, space="DRAM") as dram,
        ):
            w_sb = big.tile([128, NG * CO], F32, tag="w")
            xz_sb = big.tile([128, 8 * NG * BS], F32, tag="xz")
            xd_sb = big.tile([128, NG * BS], F32, tag="xd")
            w_in_v = w_in[:].rearrange("p (g co) -> p g co", g=NG)
            xz_in_v = xz_in[:].rearrange("p (r gb) -> p r gb", r=8)
            for g in range(NG):
                nc.sync.dma_start(
                    w_sb[:].rearrange("p (g co) -> p g co", g=NG)[:, g, :],
                    w_in_v[:, g, :])
            for r in range(8):
                nc.sync.dma_start(
                    xz_sb[:].rearrange("p (r gb) -> p r gb", r=8)[:, r, :],
                    xz_in_v[:, r, :])
            nc.sync.dma_start(xd_sb[:], xd_in[:])

            w_v = w_sb[:].rearrange("p (g co) -> p g co", g=NG)
            xd_v = xd_sb[:].rearrange("p (g b) -> p g b", g=NG)
            xz_v = xz_sb[:].rearrange("p (r g b) -> p r g b", r=8, g=NG)

            # ones pattern for denominator: [128=(c_sub 8, o 16), 16]: 1 iff p%16==m
            ones_oc = small.tile([128, OL], FP16, tag="ones")
            nc.sync.dma_start(ones_oc[:], ones_in[:])

            bias5 = small.tile([128, 1], F32, tag="bias5")
            nc.vector.memset(bias5[:], -5.0)
            s_loc = small.tile([128, 8 * BS], F32, tag="sloc")
            sr_sb = small.tile([128, 8 * BS], F32, tag="sr")
            O_sb = small.tile([128, 8 * BS], F32, tag="O")
            invO = small.tile([128, 8 * BS], F32, tag="invO")
            inv_sb = small.tile([16, NB_G * ILOC], F32, tag="inv")
            inv_bf = small.tile([16, NB_G * ILOC], FP16, tag="invbf")
            inv_rep = small.tile([128, NB_G * ILOC], FP16, tag="invrep")
            Ot_sb = small.tile([128, 8 * BS], F32, tag="Ot")
            msk = small.tile([128, 8 * BS], mybir.dt.uint8, tag="msk")
            O_sc = small.tile([128, 8 * BS], F32, tag="Osc")
            t1 = small.tile([128, 8 * BS], F32, tag="t1")
            t2 = small.tile([128, 8 * BS], F32, tag="t2")
            t3 = small.tile([128, 8 * BS], F32, tag="t3")

            cc_in = dram.tile([CO, BS], F32)
            cc_out = dram.tile([CO, BS], F32)

            def allreduce_s():
                nc.gpsimd.dma_start(
                    cc_in[:].rearrange("(ch p) b -> p ch b", p=128),
                    s_loc[:].rearrange("p (ch b) -> p ch b", ch=8),
                )
                nc.gpsimd.collective_compute(
                    "AllReduce",
                    AO.add,
                    replica_groups=[list(range(ncores))],
                    ins=[cc_in.opt()],
                    outs=[cc_out.opt()],
                )
                nc.gpsimd.dma_start(
                    sr_sb[:].rearrange("p (ch b) -> p ch b", ch=8),
                    cc_out[:].rearrange("(ch p) b -> p ch b", p=128),
                )

            def squash_and_accum(first: bool, last: bool):
                """sr_sb -> out_t = squash(sr); O += out_t; invO = O/(O^2+eps2).
                On last iter: DMA out_t to output."""
                V = sr_sb[:]
                # sq = s*s
                nc.vector.tensor_tensor(t1[:], V, V, op=AO.mult)
                # ab = max(s,0) - min(s,0)
                nc.vector.tensor_scalar(out=t2[:], in0=V, scalar1=0.0, scalar2=None,
                                        op0=AO.max)
                nc.vector.tensor_scalar(out=t3[:], in0=V, scalar1=0.0, scalar2=None,
                                        op0=AO.min)
                nc.vector.tensor_tensor(t2[:], t2[:], t3[:], op=AO.subtract)
                # den = (ab+eps)*(1+sq)
                nc.vector.tensor_scalar(out=t1[:], in0=t1[:], scalar1=1.0,
                                        scalar2=None, op0=AO.add)
                nc.vector.scalar_tensor_tensor(
                    t2[:], t2[:], EPS, t1[:], op0=AO.add, op1=AO.mult)
                nc.scalar.activation(t3[:], t2[:],
                                     mybir.ActivationFunctionType.Ln)
                nc.scalar.activation(t2[:], t3[:],
                                     mybir.ActivationFunctionType.Exp, scale=-1.0)
                # out_t = s*sq_orig*(1/den); recompute sq into t1
                nc.vector.tensor_tensor(t1[:], V, V, op=AO.mult)
                nc.vector.tensor_tensor(t2[:], t1[:], t2[:], op=AO.mult)
                nc.vector.tensor_tensor(t2[:], t2[:], V, op=AO.mult)  # out_t
                if last:
                    # out[b, c, o] <- t2[p=(cs*16+o), (ch, b)]; dst off =128*ch+p
                    ov = out_d[:].rearrange("b c o -> (c o) b").rearrange(
                        "(ch p) b -> ch p b", p=128)
                    tv = t2[:].rearrange("p (ch b) -> ch p b", ch=8)
                    for ch in range(8):
                        nc.gpsimd.dma_start(ov[ch], tv[ch])
                    return
                if first:
                    nc.vector.tensor_copy(O_sb[:], t2[:])
                else:
                    nc.vector.tensor_tensor(O_sb[:], O_sb[:], t2[:], op=AO.add)
                # Ot = sign(O)*max(|O|, TH): clamp away tiny O
                nc.vector.tensor_scalar(out=t1[:], in0=O_sb[:], scalar1=TH,
                                        scalar2=None, op0=AO.max)
                nc.vector.tensor_scalar(out=t3[:], in0=O_sb[:], scalar1=-TH,
                                        scalar2=None, op0=AO.min)
                nc.vector.tensor_scalar(out=msk[:], in0=O_sb[:], scalar1=0.0,
                                        scalar2=None, op0=AO.is_ge)
                nc.vector.select(Ot_sb[:], msk[:], t1[:], t3[:])
                nc.vector.tensor_scalar(out=O_sc[:], in0=Ot_sb[:], scalar1=SCALE,
                                        scalar2=None, op0=AO.mult)
                # invO = Ot/(Ot^2+eps2)/SCALE
                nc.vector.tensor_tensor(t1[:], Ot_sb[:], Ot_sb[:], op=AO.mult)
                nc.vector.tensor_scalar(out=t1[:], in0=t1[:], scalar1=EPS2,
                                        scalar2=None, op0=AO.add)
                nc.scalar.activation(t2[:], t1[:],
                                     mybir.ActivationFunctionType.Ln)
                nc.scalar.activation(t1[:], t2[:],
                                     mybir.ActivationFunctionType.Exp, scale=-1.0)
                nc.vector.tensor_tensor(t1[:], Ot_sb[:], t1[:], op=AO.mult)
                nc.vector.tensor_scalar(out=invO[:], in0=t1[:],
                                        scalar1=1.0 / SCALE, scalar2=None,
                                        op0=AO.mult)

            # ---------------- iter 0: s0 = (1/64) * sum_i u_hat ----------------
            for ch in range(8):
                ps0 = psu_p.tile([128, 1024], F32, tag="psu")
                for g in range(NG):
                    nc.tensor.matmul(
                        ps0[:, :BS],
                        w_v[:, g, ch * 128:(ch + 1) * 128],
                        xd_v[:, g, :],
                        start=(g == 0),
                        stop=(g == NG - 1),
                    )
                nc.vector.tensor_scalar(
                    out=s_loc[:].rearrange("p (ch b) -> p ch b", ch=8)[:, ch, :],
                    in0=ps0[:, :BS], scalar1=1.0 / NC_, scalar2=None, op0=AO.mult)
            allreduce_s()
            squash_and_accum(first=True, last=False)

            # ---------------- iters 1, 2 ----------------
            for it in (1, 2):
                for G in range(8):
                    e_tiles = []
                    l_tiles = []
                    psd = psd_p.tile([16, NB_G * ILOC], F32, tag="psd")
                    for ch in range(8):
                        l_t = lpool.tile([128, NB_G * ILOC], FP16, tag="l")
                        e_t = epool.tile([128, NB_G * ILOC], FP16, tag="e")
                        l_tiles.append(l_t)
                        e_tiles.append(e_t)
                        for half in range(2):
                            psu = psu_p.tile([128, 1024], F32, tag="psu")
                            # 128 i-values: i = 16*g + 8*j + r, half selects g>=8?
                            # half h covers i in [128h, 128h+128): g in [8h, 8h+8)
                            for gg in range(8):
                                g = 8 * half + gg
                                for j in range(2):
                                    lhsT = w_sb[64 * j:64 * j + 64, :].rearrange(
                                        "p (g co) -> p g co", g=NG
                                    )[:, g, ch * 128:(ch + 1) * 128]
                                    for r in range(8):
                                        ii = 16 * gg + 8 * j + r  # 0..127 in half
                                        nc.tensor.matmul(
                                            psu[:, ii * 8:(ii + 1) * 8],
                                            lhsT,
                                            xz_v[64 * j:64 * j + 64, r, g,
                                                 G * 8:G * 8 + 8],
                                            start=True, stop=True,
                                        )
                            # l = psu * O  (transposing write: (i,b) -> (b,i))
                            psu_v = psu[:].rearrange("p (i b) -> p b i", b=8)
                            o_bc = (
                                O_sc[:]
                                .rearrange("p (ch b) -> p ch b", ch=8)
                                [:, ch, G * 8:G * 8 + 8]
                                .unsqueeze(-1)
                                .broadcast_to([128, 8, 128])
                            )
                            l_dst = l_t[:].rearrange(
                                "p (b i) -> p b i", b=8
                            )[:, :, half * 128:half * 128 + 128]
                            nc.vector.tensor_tensor(l_dst, psu_v, o_bc, op=AO.mult)
                        # e = exp(l)
                        nc.scalar.activation(
                            e_t[:], l_t[:], mybir.ActivationFunctionType.Exp,
                            bias=bias5[:], scale=1.0 / SCALE)
                        # denominator partial: psd += ones^T @ e
                        for mm in range(4):
                            nc.tensor.matmul(
                                psd[:, mm * 512:(mm + 1) * 512],
                                ones_oc[:],
                                e_t[:, mm * 512:(mm + 1) * 512],
                                start=(ch == 0), stop=(ch == 7),
                            )
                    # inv = 1/denominator, replicate over c
                    nc.scalar.activation(inv_sb[:], psd[:],
                                          mybir.ActivationFunctionType.Ln)
                    nc.scalar.activation(inv_bf[:], inv_sb[:],
                                          mybir.ActivationFunctionType.Exp,
                                          scale=-1.0)
                    for cs in range(8):
                        nc.gpsimd.dma_start(inv_rep[16 * cs:16 * cs + 16, :],
                                            inv_bf[:])
                    for ch in range(8):
                        e_t, l_t = e_tiles[ch], l_tiles[ch]
                        # e_hat = e * inv_rep (in place)
                        nc.vector.tensor_tensor(e_t[:], e_t[:], inv_rep[:],
                                                op=AO.mult)
                        # s[co, b] = invO * sum_i e_hat*l
                        for b in range(NB_G):
                            sc = scr.tile([128, ILOC], F32, tag="scr")
                            nc.vector.scalar_tensor_tensor(
                                sc[:],
                                e_t[:, b * ILOC:(b + 1) * ILOC],
                                invO[:].rearrange("p (ch b) -> p ch b", ch=8)
                                [:, ch, G * 8 + b:G * 8 + b + 1],
                                l_t[:, b * ILOC:(b + 1) * ILOC],
                                op0=AO.mult, op1=AO.mult,
                                accum_out=s_loc[:].rearrange(
                                    "p (ch b) -> p ch b", ch=8)
                                [:, ch, G * 8 + b:G * 8 + b + 1],
                            )
                allreduce_s()
                squash_and_accum(first=False, last=(it == 2))

    nc.compile()
    return nc


def _prep_inputs(inputs, W):
    """Slice + relayout per core. Returns in_maps list."""
    x = np.asarray(inputs, dtype=np.float32)
    W = np.asarray(W, dtype=np.float32)
    in_maps = []
    for k in range(NCORES):
        i0 = k * ILOC
        Wk = W[i0:i0 + ILOC]                      # [256, 64, 16, 8]
        # -> [g16, j2, r8, c64, o16, v8] -> (j, r, v, g, c, o) -> [128, 16*1024]
        Wr = Wk.reshape(NG, 2, 8, NC_, OL, NV)
        Wc = np.ascontiguousarray(
            Wr.transpose(1, 2, 5, 0, 3, 4)).reshape(128, NG * CO)
        xk = x[:, i0:i0 + ILOC, :]                # [64, 256, 8]
        xr = xk.reshape(BS, NG, 2, 8, NV)          # b g j r v
        xdc = np.ascontiguousarray(
            xr.transpose(2, 3, 4, 1, 0)).reshape(128, NG * BS)
        rband = (np.arange(128) % 64) // 8         # r index per partition
        xzc = np.zeros((128, 8, NG * BS), np.float32)
        xzc[np.arange(128), rband, :] = xdc
        xzc = np.ascontiguousarray(xzc.reshape(128, 8 * NG * BS))
        import ml_dtypes
        ones = np.zeros((128, OL), np.float16)
        ones[np.arange(128), np.arange(128) % OL] = 1
        in_maps.append({"w": Wc, "xd": xdc, "xz": xzc, "ones": ones})
    return in_maps


def kernel(inputs, W):
    from concourse import bass_utils

    if "nc" not in _CACHE:
        _CACHE["nc"] = _build_program()
    nc = _CACHE["nc"]
    in_maps = _prep_inputs(inputs, W)
    res = bass_utils.run_bass_kernel_spmd(nc, in_maps, list(range(NCORES)))
    out = res.results[0]["out"]
    return np.asarray(out, dtype=np.float32).reshape(BS, NC_, OL)



# revision 5
# speedup vs baseline: 1.3784x; 1.3784x over previous
"""DigitCaps routing kernel v3 for 8 Trainium2 NeuronCores.

Math (reference, b-logits start at 0; O_t = sum_{tau<t} out_tau):
  u_hat[b,i,c,o] = sum_v W[i,c,o,v] * x[b,i,v]
  logits_t = u_hat * O_t (elementwise); c = softmax over capsule axis c
  s_t[b,c,o] = sum_i c * u_hat ; out_t = squash(s_t) elementwise
Identity used: s = invO * sum_i softmax(l) * l, l = u_hat*Ot,
  invO = Ot/(Ot^2+eps2), Ot = sign(O)*max(|O|,TH).

Sharding: i (2048) split across 8 cores (256 each); 3 AllReduces of
s [1024, 64] fp32.

Engine assignment per routing iter (per core, from TimelineSim ubench):
  PE : u_hat matmuls fp16 K=128/N=128 (one-hot r-packing), denominator
       psd += ones^T @ e (fp16 N=512)          ~175us
  DVE: l = psu * O_bc (fp32 PSUM in), q = e*l   ~225us
  ACT: e = exp(l) big-tile, inv = exp(-ln(psd)) ~175us
  Pool: s-accum stt (q*invO)*inv_rep -> s cols  ~231us

Per-core layouts (host-prepared, fp16):
  w16 [128, 16*1024]: w16[64j+8r+v, (g, c*16+o)] = W[i0+16g+8j+r, c, o, v]
  xd16 [128, 16*64]:  xd[64j+8r+v, (g, b)] = x[b, i0+16g+8j+r, v]
  xz4 [128, 16*64*16]: xz4[64j+8r+v, (g, b, 8j'+r')] =
      x[b, i0+16g+8j'+r', v] if (j,r)==(j',r') else 0
  ones_oc [128, 16]: 1 iff p%16 == m  (sums capsule groups -> psd[o,...])
"""
import sys

sys.path.insert(0, "/opt/trn_rl_repo")

import numpy as np

_CACHE = {}

BS, NI, NC_, OL, NV = 64, 2048, 64, 16, 8
CO = NC_ * OL          # 1024
NCORES = 8
ILOC = NI // NCORES    # 256
NG = 16                # g groups per core (16 i each)
EPS = 1e-9
EPS2 = 1e-30
TH = 1e-4
SCALE = 1024.0
_MM16 = True


def _build_program(ncores=NCORES, skip_cc=False, dbg=False, mmdt16=True):
    import concourse.bass as bass
    import concourse.bacc as bacc
    import concourse.mybir as mybir
    import concourse.tile as tile

    F32 = mybir.dt.float32
    F16 = mybir.dt.float16
    AO = mybir.AluOpType
    AF = mybir.ActivationFunctionType

    nc = bacc.Bacc("TRN2", target_bir_lowering=False, debug=False,
                   num_devices=ncores)

    MDT = F16 if mmdt16 else F32
    w_in = nc.dram_tensor("w", [128, NG * CO], MDT, kind="ExternalInput")
    xd_in = nc.dram_tensor("xd", [128, NG * BS], MDT, kind="ExternalInput")
    xz_in = nc.dram_tensor("xz", [128, NG * BS * 16], MDT, kind="ExternalInput")
    ones_in = nc.dram_tensor("ones", [128, OL], F16, kind="ExternalInput")
    out_d = nc.dram_tensor("out", [BS, NC_, OL], F32, kind="ExternalOutput")
    if dbg:
        dbg_s = [nc.dram_tensor(f"dbg_s{t}", [128, 8 * BS], F32,
                                kind="ExternalOutput") for t in range(3)]
        dbg_O = [nc.dram_tensor(f"dbg_O{t}", [128, 8 * BS], F32,
                                kind="ExternalOutput") for t in range(2)]
        dbg_q = nc.dram_tensor("dbg_q", [128, 8 * ILOC], F16,
                               kind="ExternalOutput")
        dbg_inv = nc.dram_tensor("dbg_inv", [128, 8 * ILOC], F16,
                                 kind="ExternalOutput")

    with tile.TileContext(nc) as tc:
        with (
            tc.tile_pool(name="big", bufs=1) as big,
            tc.tile_pool(name="lpool", bufs=12) as lpool,
            tc.tile_pool(name="epool", bufs=12) as epool,
            tc.tile_pool(name="small", bufs=1) as small,
            tc.tile_pool(name="invp", bufs=2) as invp,
            tc.tile_pool(name="inv1", bufs=1) as inv1,
            tc.tile_pool(name="scr", bufs=4) as scr,
            tc.tile_pool(name="psu_p", bufs=2, space="PSUM") as psu_p,
            tc.tile_pool(name="psd_p", bufs=1, space="PSUM") as psd_p,
            tc.tile_pool(name="dram", bufs=2, space="DRAM") as dram,
        ):
            w_sb = big.tile([128, NG * CO], MDT, tag="w")
            xz_sb = big.tile([128, NG * BS * 16], MDT, tag="xz")
            xd_sb = big.tile([128, NG * BS], MDT, tag="xd")
            w_in_v = w_in[:].rearrange("p (g co) -> p g co", g=NG)
            xz_in_v = xz_in[:].rearrange("p (g c) -> p g c", g=NG)
            # w + xd first (iter 0 needs them); xz only needed at iter 1
            nc.sync.dma_start(xd_sb[:], xd_in[:])
            for g in range(NG):
                nc.sync.dma_start(
                    w_sb[:].rearrange("p (g co) -> p g co", g=NG)[:, g, :],
                    w_in_v[:, g, :])
            ones_oc = small.tile([128, OL], F16, tag="ones")
            nc.sync.dma_start(ones_oc[:], ones_in[:])
            for g in range(NG):
                nc.sync.dma_start(
                    xz_sb[:].rearrange("p (g c) -> p g c", g=NG)[:, g, :],
                    xz_in_v[:, g, :])

            w_v = w_sb[:].rearrange("p (g co) -> p g co", g=NG)
            xd_v = xd_sb[:].rearrange("p (g b) -> p g b", g=NG)
            xz_v = xz_sb[:].rearrange("p (g b jr) -> p g b jr", g=NG, b=BS)

            bias5 = small.tile([128, 1], F32, tag="bias5")
            nc.vector.memset(bias5[:], -5.0)
            s_loc = small.tile([128, 8 * BS], F32, tag="sloc")
            sr_sb = small.tile([128, 8 * BS], F32, tag="sr")
            O_sb = small.tile([128, 8 * BS], F32, tag="O")
            Ot_sb = small.tile([128, 8 * BS], F32, tag="Ot")
            O_sc = small.tile([128, 8 * BS], F32, tag="Osc")
            invO = small.tile([128, 8 * BS], F32, tag="invO")
            msk = small.tile([128, 8 * BS], mybir.dt.uint8, tag="msk")
            t1 = small.tile([128, 8 * BS], F32, tag="t1")
            t2 = small.tile([128, 8 * BS], F32, tag="t2")
            t3 = small.tile([128, 8 * BS], F32, tag="t3")

            cc_in = dram.tile([CO, BS], F32)
            cc_out = dram.tile([CO, BS], F32)

            arc = [0]

            def allreduce_s():
                nc.gpsimd.dma_start(
                    cc_in[:].rearrange("(ch p) b -> p ch b", p=128),
                    s_loc[:].rearrange("p (ch b) -> p ch b", ch=8),
                )
                if skip_cc:
                    nc.gpsimd.dma_start(cc_out[:], cc_in[:])
                else:
                    nc.gpsimd.collective_compute(
                        "AllReduce",
                        AO.add,
                        replica_groups=[list(range(ncores))],
                        ins=[cc_in.opt()],
                        outs=[cc_out.opt()],
                    )
                nc.gpsimd.dma_start(
                    sr_sb[:].rearrange("p (ch b) -> p ch b", ch=8),
                    cc_out[:].rearrange("(ch p) b -> p ch b", p=128),
                )
                if dbg:
                    nc.sync.dma_start(dbg_s[arc[0]][:], sr_sb[:])
                arc[0] += 1

            def squash_and_accum(first: bool, last: bool):
                """sr_sb -> out_t = squash(sr); O += out_t;
                Ot = clamp(O); invO = Ot/(Ot^2+eps2).
                On last iter: DMA out_t to output."""
                V = sr_sb[:]
                # sq = s*s
                nc.vector.tensor_tensor(t1[:], V, V, op=AO.mult)
                # ab = max(s,0) - min(s,0) = |s|
                nc.vector.tensor_scalar(out=t2[:], in0=V, scalar1=0.0,
                                        scalar2=None, op0=AO.max)
                nc.vector.tensor_scalar(out=t3[:], in0=V, scalar1=0.0,
                                        scalar2=None, op0=AO.min)
                nc.vector.tensor_tensor(t2[:], t2[:], t3[:], op=AO.subtract)
                # den = (ab+eps)*(1+sq); t2 = 1/den via exp(-ln(den))
                nc.vector.tensor_scalar(out=t1[:], in0=t1[:], scalar1=1.0,
                                        scalar2=None, op0=AO.add)
                nc.vector.scalar_tensor_tensor(
                    t2[:], t2[:], EPS, t1[:], op0=AO.add, op1=AO.mult)
                nc.scalar.activation(t3[:], t2[:], AF.Ln)
                nc.scalar.activation(t2[:], t3[:], AF.Exp, scale=-1.0)
                # out_t = s*sq*(1/den); recompute sq into t1
                nc.vector.tensor_tensor(t1[:], V, V, op=AO.mult)
                nc.vector.tensor_tensor(t2[:], t1[:], t2[:], op=AO.mult)
                nc.vector.tensor_tensor(t2[:], t2[:], V, op=AO.mult)  # out_t
                if last:
                    # out[b, c, o] <- t2[p=(cs*16+o), (ch, b)]
                    ov = out_d[:].rearrange("b c o -> (c o) b").rearrange(
                        "(ch p) b -> ch p b", p=128)
                    tv = t2[:].rearrange("p (ch b) -> ch p b", ch=8)
                    for ch in range(8):
                        nc.gpsimd.dma_start(ov[ch], tv[ch])
                    return
                if first:
                    nc.vector.tensor_copy(O_sb[:], t2[:])
                else:
                    nc.vector.tensor_tensor(O_sb[:], O_sb[:], t2[:], op=AO.add)
                if dbg:
                    nc.sync.dma_start(dbg_O[arc[0] - 1][:], O_sb[:])
                # Ot = sign(O)*max(|O|, TH)
                nc.vector.tensor_scalar(out=t1[:], in0=O_sb[:], scalar1=TH,
                                        scalar2=None, op0=AO.max)
                nc.vector.tensor_scalar(out=t3[:], in0=O_sb[:], scalar1=-TH,
                                        scalar2=None, op0=AO.min)
                nc.vector.tensor_scalar(out=msk[:], in0=O_sb[:], scalar1=0.0,
                                        scalar2=None, op0=AO.is_ge)
                nc.vector.select(Ot_sb[:], msk[:], t1[:], t3[:])
                nc.vector.tensor_scalar(out=O_sc[:], in0=Ot_sb[:],
                                        scalar1=SCALE, scalar2=None,
                                        op0=AO.mult)
                # invO = Ot/(Ot^2+eps2) via exp(-ln(Ot^2+eps2))*Ot
                nc.vector.tensor_tensor(t1[:], Ot_sb[:], Ot_sb[:], op=AO.mult)
                nc.vector.tensor_scalar(out=t1[:], in0=t1[:], scalar1=EPS2,
                                        scalar2=None, op0=AO.add)
                nc.scalar.activation(t2[:], t1[:], AF.Ln)
                nc.scalar.activation(t1[:], t2[:], AF.Exp, scale=-1.0)
                nc.vector.scalar_tensor_tensor(invO[:], Ot_sb[:],
                                               1.0 / SCALE, t1[:],
                                               op0=AO.mult, op1=AO.mult)

            # ---------------- iter 0: s0 = (1/64) * sum_i u_hat -------------
            for ch in range(8):
                ps0_full = psu_p.tile([128, 1024], F32, tag="psu")
                ps0 = ps0_full[:, :BS]
                for g in range(NG):
                    nc.tensor.matmul(
                        ps0,
                        w_v[:, g, ch * 128:(ch + 1) * 128],
                        xd_v[:, g, :],
                        start=(g == 0),
                        stop=(g == NG - 1),
                    )
                nc.vector.tensor_scalar(
                    out=s_loc[:].rearrange("p (ch b) -> p ch b", ch=8)[:, ch, :],
                    in0=ps0[:], scalar1=1.0 / NC_, scalar2=None, op0=AO.mult)
            allreduce_s()
            squash_and_accum(first=True, last=False)

            # ---------------- iters 1, 2 ----------------
            Ot_v = O_sc[:].rearrange("p (ch b) -> p ch b", ch=8)
            invO_v = invO[:].rearrange("p (ch b) -> p ch b", ch=8)
            sl_v = s_loc[:].rearrange("p (ch b) -> p ch b", ch=8)
            NACT = 3   # ch-units routed via ACT accumulate (ch >= 8-NACT)

            def stt_batch(G, q_tiles, l_tiles, inv_rep):
                """ehat = e*inv_rep (normalized weights <= 1, fp16-safe),
                then s[co, b] = sum_i (ehat * invO) * l."""
                for ch in range(8):
                    # ehat = e * inv_rep in-place (Pool)
                    nc.gpsimd.tensor_tensor(q_tiles[ch][:], q_tiles[ch][:],
                                            inv_rep[:], op=AO.mult)
                for ch in range(8 - NACT, 8):
                    # q2 = ehat * l for the ACT-accum route (DVE)
                    nc.vector.tensor_tensor(q_tiles[ch][:], q_tiles[ch][:],
                                            l_tiles[ch][:], op=AO.mult)
                for ch in range(8):
                    q_t, l_t = q_tiles[ch], l_tiles[ch]
                    for b in range(8):
                        sc = scr.tile([128, ILOC], F16, tag="scr")
                        if ch < 8 - NACT:
                            nc.vector.scalar_tensor_tensor(
                                sc[:],
                                q_t[:, b * ILOC:(b + 1) * ILOC],
                                invO_v[:, ch, G * 8 + b:G * 8 + b + 1],
                                l_t[:, b * ILOC:(b + 1) * ILOC],
                                op0=AO.mult, op1=AO.mult,
                                accum_out=sl_v[:, ch,
                                               G * 8 + b:G * 8 + b + 1],
                            )
                        else:
                            nc.scalar.activation(
                                sc[:],
                                q_t[:, b * ILOC:(b + 1) * ILOC],
                                AF.Copy,
                                scale=invO_v[:, ch,
                                             G * 8 + b:G * 8 + b + 1],
                                accum_out=sl_v[:, ch,
                                               G * 8 + b:G * 8 + b + 1],
                            )

            for it in (1, 2):
                pend = None
                for G in range(8):
                    psd = psd_p.tile([16, 8 * ILOC], F32, tag="psd")
                    q_tiles = []
                    l_tiles = []
                    for ch in range(8):
                        l_t = lpool.tile([128, 8 * ILOC], F16, tag="l")
                        e_t = epool.tile([128, 8 * ILOC], F16, tag="e")
                        q_tiles.append(e_t)
                        l_tiles.append(l_t)
                        l_v = l_t[:].rearrange("p (b i) -> p b i", b=8)
                        o_bc = (Ot_v[:, ch, G * 8:G * 8 + 8]
                                .unsqueeze(-1).broadcast_to([128, 8, 128]))
                        for half in range(2):
                            # psu cols = (gg 8, b 8, jr 16): each matmul
                            # writes 128 contiguous cols inside one bank pair
                            psu = psu_p.tile([128, 1024], F32, tag="psu")
                            for gg in range(8):
                                g = 8 * half + gg
                                nc.tensor.matmul(
                                    psu[:, gg * 128:gg * 128 + 128],
                                    w_v[:, g, ch * 128:(ch + 1) * 128],
                                    xz_v[:, g, G * 8:G * 8 + 8, :],
                                    start=True, stop=True,
                                )
                            # l[b, i=half*128+gg*16+jr] = psu[(gg,b,jr)] * Ot
                            psu_v = psu[:].rearrange(
                                "p (gg b jr) -> p b gg jr", gg=8, b=8)
                            l_d = (l_t[:]
                                   .rearrange("p (b g2 jr) -> p b g2 jr",
                                              b=8, g2=16)
                                   [:, :, half * 8:half * 8 + 8, :])
                            o_bc4 = (Ot_v[:, ch, G * 8:G * 8 + 8]
                                     .unsqueeze(-1).unsqueeze(-1)
                                     .broadcast_to([128, 8, 8, 16]))
                            nc.vector.tensor_tensor(l_d, psu_v, o_bc4,
                                                    op=AO.mult)
                        # e = exp(l - 5): keeps 1/psd inside fp16 normal
                        # range (the e^5 factor cancels num/denom exactly)
                        nc.scalar.activation(e_t[:], l_t[:], AF.Exp,
                                             bias=bias5[:],
                                             scale=1.0 / SCALE)
                        # denominator partial: psd += ones^T @ e
                        for mm in range(4):
                            nc.tensor.matmul(
                                psd[:, mm * 512:(mm + 1) * 512],
                                ones_oc[:],
                                e_t[:, mm * 512:(mm + 1) * 512],
                                start=(ch == 0), stop=(ch == 7),
                            )
                    # inv = 1/psd via exp(-ln); replicate over c
                    it1 = inv1.tile([16, 8 * ILOC], F32, tag="invt")
                    inv_bf = inv1.tile([16, 8 * ILOC], F16, tag="invbf")
                    inv_rep = invp.tile([128, 8 * ILOC], F16, tag="invrep")
                    nc.scalar.activation(it1[:], psd[:], AF.Ln)
                    nc.scalar.activation(inv_bf[:], it1[:], AF.Exp, scale=-1.0)
                    for cs in range(8):
                        nc.sync.dma_start(inv_rep[16 * cs:16 * cs + 16, :],
                                          inv_bf[:])
                    if dbg and it == 1 and G == 0:
                        nc.sync.dma_start(dbg_q[:], q_tiles[0][:])
                        nc.sync.dma_start(dbg_inv[:], inv_rep[:])
                    if pend is not None:
                        stt_batch(*pend)
                    pend = (G, q_tiles, l_tiles, inv_rep)
                stt_batch(*pend)
                allreduce_s()
                squash_and_accum(first=False, last=(it == 2))

    nc.compile()
    return nc


def _prep_inputs(inputs, W):
    """Slice + relayout per core (fp16). Returns in_maps list."""
    x = np.asarray(inputs, dtype=np.float32)
    W = np.asarray(W, dtype=np.float32)
    in_maps = []
    ones = np.zeros((128, OL), np.float16)
    ones[np.arange(128), np.arange(128) % OL] = 1
    for k in range(NCORES):
        i0 = k * ILOC
        Wk = W[i0:i0 + ILOC]                      # [256, 64, 16, 8]
        # -> [g16, j2, r8, c64, o16, v8] -> (j, r, v, g, c, o)
        Wr = Wk.reshape(NG, 2, 8, NC_, OL, NV)
        Wc = np.ascontiguousarray(
            Wr.transpose(1, 2, 5, 0, 3, 4)).reshape(128, NG * CO)
        xk = x[:, i0:i0 + ILOC, :]                # [64, 256, 8]
        xr = xk.reshape(BS, NG, 2, 8, NV)          # b g j r v
        xdc = np.ascontiguousarray(
            xr.transpose(2, 3, 4, 1, 0)).reshape(128, NG * BS)
        # xz4[64j+8r+v, (g, b, jr')] = xdc[64j+8r+v, (g, b)] iff jr'==8j+r
        p = np.arange(128)
        jrband = 8 * (p // 64) + (p % 64) // 8     # 8j+r per partition (0..15)
        xz4 = np.zeros((128, NG * BS, 16), np.float32)
        xz4[p, :, jrband] = xdc
        mdt = np.float16 if _MM16 else np.float32
        in_maps.append({
            "w": Wc.astype(mdt),
            "xd": xdc.astype(mdt),
            "xz": xz4.reshape(128, NG * BS * 16).astype(mdt),
            "ones": ones,
        })
    return in_maps


def kernel(inputs, W):
    from concourse import bass_utils

    if "nc" not in _CACHE:
        _CACHE["nc"] = _build_program(mmdt16=_MM16)
    nc = _CACHE["nc"]
    in_maps = _prep_inputs(inputs, W)
    res = bass_utils.run_bass_kernel_spmd(nc, in_maps, list(range(NCORES)))
    out = res.results[0]["out"]
    return np.asarray(out, dtype=np.float32).reshape(BS, NC_, OL)


# revision 6
# speedup vs baseline: 1.4451x; 1.0483x over previous
"""DigitCaps routing kernel v3 for 8 Trainium2 NeuronCores.

Math (reference, b-logits start at 0; O_t = sum_{tau<t} out_tau):
  u_hat[b,i,c,o] = sum_v W[i,c,o,v] * x[b,i,v]
  logits_t = u_hat * O_t (elementwise); c = softmax over capsule axis c
  s_t[b,c,o] = sum_i c * u_hat ; out_t = squash(s_t) elementwise
Identity used: s = invO * sum_i softmax(l) * l, l = u_hat*Ot,
  invO = Ot/(Ot^2+eps2), Ot = sign(O)*max(|O|,TH).

Sharding: i (2048) split across 8 cores (256 each); 3 AllReduces of
s [1024, 64] fp32.

Engine assignment per routing iter (per core, from TimelineSim ubench):
  PE : u_hat matmuls fp16 K=128/N=128 (one-hot r-packing), denominator
       psd += ones^T @ e (fp16 N=512)          ~175us
  DVE: l = psu * O_bc (fp32 PSUM in), q = e*l   ~225us
  ACT: e = exp(l) big-tile, inv = exp(-ln(psd)) ~175us
  Pool: s-accum stt (q*invO)*inv_rep -> s cols  ~231us

Per-core layouts (host-prepared, fp16):
  w16 [128, 16*1024]: w16[64j+8r+v, (g, c*16+o)] = W[i0+16g+8j+r, c, o, v]
  xd16 [128, 16*64]:  xd[64j+8r+v, (g, b)] = x[b, i0+16g+8j+r, v]
  xz4 [128, 16*64*16]: xz4[64j+8r+v, (g, b, 8j'+r')] =
      x[b, i0+16g+8j'+r', v] if (j,r)==(j',r') else 0
  ones_oc [128, 16]: 1 iff p%16 == m  (sums capsule groups -> psd[o,...])
"""
import sys

sys.path.insert(0, "/opt/trn_rl_repo")

import numpy as np

_CACHE = {}

BS, NI, NC_, OL, NV = 64, 2048, 64, 16, 8
CO = NC_ * OL          # 1024
NCORES = 8
ILOC = NI // NCORES    # 256
NG = 16                # g groups per core (16 i each)
EPS = 1e-9
EPS2 = 1e-30
TH = 1e-4
SCALE = 1024.0
_MM16 = True


def _build_program(ncores=NCORES, skip_cc=False, dbg=False, mmdt16=True):
    import concourse.bass as bass
    import concourse.bacc as bacc
    import concourse.mybir as mybir
    import concourse.tile as tile

    F32 = mybir.dt.float32
    F16 = mybir.dt.float16
    AO = mybir.AluOpType
    AF = mybir.ActivationFunctionType

    nc = bacc.Bacc("TRN2", target_bir_lowering=False, debug=False,
                   num_devices=ncores)

    MDT = F16 if mmdt16 else F32
    w_in = nc.dram_tensor("w", [128, NG * CO], MDT, kind="ExternalInput")
    xd_in = nc.dram_tensor("xd", [128, NG * BS], MDT, kind="ExternalInput")
    xz_in = nc.dram_tensor("xz", [128, NG * BS * 16], MDT, kind="ExternalInput")
    ones_in = nc.dram_tensor("ones", [128, OL], F16, kind="ExternalInput")
    out_d = nc.dram_tensor("out", [BS, NC_, OL], F32, kind="ExternalOutput")
    if dbg:
        dbg_s = [nc.dram_tensor(f"dbg_s{t}", [128, 8 * BS], F32,
                                kind="ExternalOutput") for t in range(3)]
        dbg_O = [nc.dram_tensor(f"dbg_O{t}", [128, 8 * BS], F32,
                                kind="ExternalOutput") for t in range(2)]
        dbg_q = nc.dram_tensor("dbg_q", [128, 8 * ILOC], F16,
                               kind="ExternalOutput")
        dbg_inv = nc.dram_tensor("dbg_inv", [128, 8 * ILOC], F16,
                                 kind="ExternalOutput")

    with tile.TileContext(nc) as tc:
        with (
            tc.tile_pool(name="big", bufs=1) as big,
            tc.tile_pool(name="lpool", bufs=12) as lpool,
            tc.tile_pool(name="epool", bufs=12) as epool,
            tc.tile_pool(name="small", bufs=1) as small,
            tc.tile_pool(name="invp", bufs=2) as invp,
            tc.tile_pool(name="inv1", bufs=1) as inv1,
            tc.tile_pool(name="scr", bufs=8) as scr,
            tc.tile_pool(name="scra", bufs=8) as scra,
            tc.tile_pool(name="psu_p", bufs=2, space="PSUM") as psu_p,
            tc.tile_pool(name="psd_p", bufs=1, space="PSUM") as psd_p,
            tc.tile_pool(name="dram", bufs=2, space="DRAM") as dram,
        ):
            w_sb = big.tile([128, NG * CO], MDT, tag="w")
            xz_sb = big.tile([128, NG * BS * 16], MDT, tag="xz")
            xd_sb = big.tile([128, NG * BS], MDT, tag="xd")
            w_in_v = w_in[:].rearrange("p (g co) -> p g co", g=NG)
            xz_in_v = xz_in[:].rearrange("p (g c) -> p g c", g=NG)
            # w + xd first (iter 0 needs them); xz only needed at iter 1
            nc.sync.dma_start(xd_sb[:], xd_in[:])
            for g in range(NG):
                nc.sync.dma_start(
                    w_sb[:].rearrange("p (g co) -> p g co", g=NG)[:, g, :],
                    w_in_v[:, g, :])
            ones_oc = small.tile([128, OL], F16, tag="ones")
            nc.sync.dma_start(ones_oc[:], ones_in[:])
            for g in range(NG):
                nc.sync.dma_start(
                    xz_sb[:].rearrange("p (g c) -> p g c", g=NG)[:, g, :],
                    xz_in_v[:, g, :])

            w_v = w_sb[:].rearrange("p (g co) -> p g co", g=NG)
            xd_v = xd_sb[:].rearrange("p (g b) -> p g b", g=NG)
            xz_v = xz_sb[:].rearrange("p (g b jr) -> p g b jr", g=NG, b=BS)

            bias5 = small.tile([128, 1], F32, tag="bias5")
            nc.vector.memset(bias5[:], -5.0)
            s_loc = small.tile([128, 8 * BS], F32, tag="sloc")
            sr_sb = small.tile([128, 8 * BS], F32, tag="sr")
            O_sb = small.tile([128, 8 * BS], F32, tag="O")
            Ot_sb = small.tile([128, 8 * BS], F32, tag="Ot")
            O_sc = small.tile([128, 8 * BS], F32, tag="Osc")
            invO = small.tile([128, 8 * BS], F32, tag="invO")
            msk = small.tile([128, 8 * BS], mybir.dt.uint8, tag="msk")
            t1 = small.tile([128, 8 * BS], F32, tag="t1")
            t2 = small.tile([128, 8 * BS], F32, tag="t2")
            t3 = small.tile([128, 8 * BS], F32, tag="t3")

            cc_in = dram.tile([CO, BS], F32)
            cc_out = dram.tile([CO, BS], F32)

            arc = [0]

            def allreduce_s():
                nc.gpsimd.dma_start(
                    cc_in[:].rearrange("(ch p) b -> p ch b", p=128),
                    s_loc[:].rearrange("p (ch b) -> p ch b", ch=8),
                )
                if skip_cc:
                    nc.gpsimd.dma_start(cc_out[:], cc_in[:])
                else:
                    nc.gpsimd.collective_compute(
                        "AllReduce",
                        AO.add,
                        replica_groups=[list(range(ncores))],
                        ins=[cc_in.opt()],
                        outs=[cc_out.opt()],
                    )
                nc.gpsimd.dma_start(
                    sr_sb[:].rearrange("p (ch b) -> p ch b", ch=8),
                    cc_out[:].rearrange("(ch p) b -> p ch b", p=128),
                )
                if dbg:
                    nc.sync.dma_start(dbg_s[arc[0]][:], sr_sb[:])
                arc[0] += 1

            def squash_and_accum(first: bool, last: bool):
                """sr_sb -> out_t = squash(sr); O += out_t;
                Ot = clamp(O); invO = Ot/(Ot^2+eps2).
                On last iter: DMA out_t to output."""
                V = sr_sb[:]
                # sq = s*s
                nc.vector.tensor_tensor(t1[:], V, V, op=AO.mult)
                # ab = max(s,0) - min(s,0) = |s|
                nc.vector.tensor_scalar(out=t2[:], in0=V, scalar1=0.0,
                                        scalar2=None, op0=AO.max)
                nc.vector.tensor_scalar(out=t3[:], in0=V, scalar1=0.0,
                                        scalar2=None, op0=AO.min)
                nc.vector.tensor_tensor(t2[:], t2[:], t3[:], op=AO.subtract)
                # den = (ab+eps)*(1+sq); t2 = 1/den via exp(-ln(den))
                nc.vector.tensor_scalar(out=t1[:], in0=t1[:], scalar1=1.0,
                                        scalar2=None, op0=AO.add)
                nc.vector.scalar_tensor_tensor(
                    t2[:], t2[:], EPS, t1[:], op0=AO.add, op1=AO.mult)
                nc.scalar.activation(t3[:], t2[:], AF.Ln)
                nc.scalar.activation(t2[:], t3[:], AF.Exp, scale=-1.0)
                # out_t = s*sq*(1/den); recompute sq into t1
                nc.vector.tensor_tensor(t1[:], V, V, op=AO.mult)
                nc.vector.tensor_tensor(t2[:], t1[:], t2[:], op=AO.mult)
                nc.vector.tensor_tensor(t2[:], t2[:], V, op=AO.mult)  # out_t
                if last:
                    # out[b, c, o] <- t2[p=(cs*16+o), (ch, b)]
                    ov = out_d[:].rearrange("b c o -> (c o) b").rearrange(
                        "(ch p) b -> ch p b", p=128)
                    tv = t2[:].rearrange("p (ch b) -> ch p b", ch=8)
                    for ch in range(8):
                        nc.gpsimd.dma_start(ov[ch], tv[ch])
                    return
                if first:
                    nc.vector.tensor_copy(O_sb[:], t2[:])
                else:
                    nc.vector.tensor_tensor(O_sb[:], O_sb[:], t2[:], op=AO.add)
                if dbg:
                    nc.sync.dma_start(dbg_O[arc[0] - 1][:], O_sb[:])
                # Ot = sign(O)*max(|O|, TH)
                nc.vector.tensor_scalar(out=t1[:], in0=O_sb[:], scalar1=TH,
                                        scalar2=None, op0=AO.max)
                nc.vector.tensor_scalar(out=t3[:], in0=O_sb[:], scalar1=-TH,
                                        scalar2=None, op0=AO.min)
                nc.vector.tensor_scalar(out=msk[:], in0=O_sb[:], scalar1=0.0,
                                        scalar2=None, op0=AO.is_ge)
                nc.vector.select(Ot_sb[:], msk[:], t1[:], t3[:])
                nc.vector.tensor_scalar(out=O_sc[:], in0=Ot_sb[:],
                                        scalar1=SCALE, scalar2=None,
                                        op0=AO.mult)
                # invO = Ot/(Ot^2+eps2) via exp(-ln(Ot^2+eps2))*Ot
                nc.vector.tensor_tensor(t1[:], Ot_sb[:], Ot_sb[:], op=AO.mult)
                nc.vector.tensor_scalar(out=t1[:], in0=t1[:], scalar1=EPS2,
                                        scalar2=None, op0=AO.add)
                nc.scalar.activation(t2[:], t1[:], AF.Ln)
                nc.scalar.activation(t1[:], t2[:], AF.Exp, scale=-1.0)
                nc.vector.scalar_tensor_tensor(invO[:], Ot_sb[:],
                                               1.0 / SCALE, t1[:],
                                               op0=AO.mult, op1=AO.mult)

            # ---------------- iter 0: s0 = (1/64) * sum_i u_hat -------------
            for ch in range(8):
                ps0_full = psu_p.tile([128, 1024], F32, tag="psu")
                ps0 = ps0_full[:, :BS]
                for g in range(NG):
                    nc.tensor.matmul(
                        ps0,
                        w_v[:, g, ch * 128:(ch + 1) * 128],
                        xd_v[:, g, :],
                        start=(g == 0),
                        stop=(g == NG - 1),
                    )
                nc.vector.tensor_scalar(
                    out=s_loc[:].rearrange("p (ch b) -> p ch b", ch=8)[:, ch, :],
                    in0=ps0[:], scalar1=1.0 / NC_, scalar2=None, op0=AO.mult)
            allreduce_s()
            squash_and_accum(first=True, last=False)

            # ---------------- iters 1, 2 ----------------
            Ot_v = O_sc[:].rearrange("p (ch b) -> p ch b", ch=8)
            invO_v = invO[:].rearrange("p (ch b) -> p ch b", ch=8)
            sl_v = s_loc[:].rearrange("p (ch b) -> p ch b", ch=8)
            NACT = 3   # ch-units routed via ACT accumulate (ch >= 8-NACT)

            def stt_batch(G, q_tiles, l_tiles, inv_rep):
                """ehat = e*inv_rep (normalized weights <= 1, fp16-safe),
                then s[co, b] = sum_i (ehat * invO) * l."""
                for ch in range(8):
                    # ehat = e * inv_rep in-place (Pool)
                    nc.gpsimd.tensor_tensor(q_tiles[ch][:], q_tiles[ch][:],
                                            inv_rep[:], op=AO.mult)
                for ch in range(8 - NACT, 8):
                    # q2 = ehat * l for the ACT-accum route (DVE)
                    nc.vector.tensor_tensor(q_tiles[ch][:], q_tiles[ch][:],
                                            l_tiles[ch][:], op=AO.mult)
                for ch in range(8):
                    q_t, l_t = q_tiles[ch], l_tiles[ch]
                    for b in range(8):
                        sc = scr.tile([128, ILOC], F16, tag="scr")
                        if ch < 8 - NACT:
                            nc.vector.scalar_tensor_tensor(
                                sc[:],
                                q_t[:, b * ILOC:(b + 1) * ILOC],
                                invO_v[:, ch, G * 8 + b:G * 8 + b + 1],
                                l_t[:, b * ILOC:(b + 1) * ILOC],
                                op0=AO.mult, op1=AO.mult,
                                accum_out=sl_v[:, ch,
                                               G * 8 + b:G * 8 + b + 1],
                            )
                        else:
                            nc.scalar.activation(
                                sc[:],
                                q_t[:, b * ILOC:(b + 1) * ILOC],
                                AF.Copy,
                                scale=invO_v[:, ch,
                                             G * 8 + b:G * 8 + b + 1],
                                accum_out=sl_v[:, ch,
                                               G * 8 + b:G * 8 + b + 1],
                            )

            for it in (1, 2):
                pend = None
                for G in range(8):
                    psd = psd_p.tile([16, 8 * ILOC], F32, tag="psd")
                    q_tiles = []
                    l_tiles = []
                    for ch in range(8):
                        l_t = lpool.tile([128, 8 * ILOC], F16, tag="l")
                        e_t = epool.tile([128, 8 * ILOC], F16, tag="e")
                        q_tiles.append(e_t)
                        l_tiles.append(l_t)
                        l_v = l_t[:].rearrange("p (b i) -> p b i", b=8)
                        o_bc = (Ot_v[:, ch, G * 8:G * 8 + 8]
                                .unsqueeze(-1).broadcast_to([128, 8, 128]))
                        for half in range(2):
                            # psu cols = (gg 8, b 8, jr 16): each matmul
                            # writes 128 contiguous cols inside one bank pair
                            psu = psu_p.tile([128, 1024], F32, tag="psu")
                            for gg in range(8):
                                g = 8 * half + gg
                                nc.tensor.matmul(
                                    psu[:, gg * 128:gg * 128 + 128],
                                    w_v[:, g, ch * 128:(ch + 1) * 128],
                                    xz_v[:, g, G * 8:G * 8 + 8, :],
                                    start=True, stop=True,
                                )
                            # l[b, i=half*128+gg*16+jr] = psu[(gg,b,jr)] * Ot
                            psu_v = psu[:].rearrange(
                                "p (gg b jr) -> p b gg jr", gg=8, b=8)
                            l_d = (l_t[:]
                                   .rearrange("p (b g2 jr) -> p b g2 jr",
                                              b=8, g2=16)
                                   [:, :, half * 8:half * 8 + 8, :])
                            o_bc4 = (Ot_v[:, ch, G * 8:G * 8 + 8]
                                     .unsqueeze(-1).unsqueeze(-1)
                                     .broadcast_to([128, 8, 8, 16]))
                            nc.vector.tensor_tensor(l_d, psu_v, o_bc4,
                                                    op=AO.mult)
                        # e = exp(l - 5): keeps 1/psd inside fp16 normal
                        # range (the e^5 factor cancels num/denom exactly)
                        nc.scalar.activation(e_t[:], l_t[:], AF.Exp,
                                             bias=bias5[:],
                                             scale=1.0 / SCALE)
                        # denominator partial: psd += ones^T @ e
                        for mm in range(4):
                            nc.tensor.matmul(
                                psd[:, mm * 512:(mm + 1) * 512],
                                ones_oc[:],
                                e_t[:, mm * 512:(mm + 1) * 512],
                                start=(ch == 0), stop=(ch == 7),
                            )
                    # inv = 1/psd via exp(-ln); replicate over c
                    it1 = inv1.tile([16, 8 * ILOC], F32, tag="invt")
                    inv_bf = inv1.tile([16, 8 * ILOC], F16, tag="invbf")
                    inv_rep = invp.tile([128, 8 * ILOC], F16, tag="invrep")
                    nc.scalar.activation(it1[:], psd[:], AF.Ln)
                    nc.scalar.activation(inv_bf[:], it1[:], AF.Exp, scale=-1.0)
                    for cs in range(8):
                        nc.sync.dma_start(inv_rep[16 * cs:16 * cs + 16, :],
                                          inv_bf[:])
                    if dbg and it == 1 and G == 0:
                        nc.sync.dma_start(dbg_q[:], q_tiles[0][:])
                        nc.sync.dma_start(dbg_inv[:], inv_rep[:])
                    if pend is not None:
                        stt_batch(*pend)
                    pend = (G, q_tiles, l_tiles, inv_rep)
                stt_batch(*pend)
                allreduce_s()
                squash_and_accum(first=False, last=(it == 2))

    nc.compile()
    return nc


def _prep_inputs(inputs, W):
    """Slice + relayout per core (fp16). Returns in_maps list."""
    x = np.asarray(inputs, dtype=np.float32)
    W = np.asarray(W, dtype=np.float32)
    in_maps = []
    ones = np.zeros((128, OL), np.float16)
    ones[np.arange(128), np.arange(128) % OL] = 1
    for k in range(NCORES):
        i0 = k * ILOC
        Wk = W[i0:i0 + ILOC]                      # [256, 64, 16, 8]
        # -> [g16, j2, r8, c64, o16, v8] -> (j, r, v, g, c, o)
        Wr = Wk.reshape(NG, 2, 8, NC_, OL, NV)
        Wc = np.ascontiguousarray(
            Wr.transpose(1, 2, 5, 0, 3, 4)).reshape(128, NG * CO)
        xk = x[:, i0:i0 + ILOC, :]                # [64, 256, 8]
        xr = xk.reshape(BS, NG, 2, 8, NV)          # b g j r v
        xdc = np.ascontiguousarray(
            xr.transpose(2, 3, 4, 1, 0)).reshape(128, NG * BS)
        # xz4[64j+8r+v, (g, b, jr')] = xdc[64j+8r+v, (g, b)] iff jr'==8j+r
        p = np.arange(128)
        jrband = 8 * (p // 64) + (p % 64) // 8     # 8j+r per partition (0..15)
        xz4 = np.zeros((128, NG * BS, 16), np.float32)
        xz4[p, :, jrband] = xdc
        mdt = np.float16 if _MM16 else np.float32
        in_maps.append({
            "w": Wc.astype(mdt),
            "xd": xdc.astype(mdt),
            "xz": xz4.reshape(128, NG * BS * 16).astype(mdt),
            "ones": ones,
        })
    return in_maps


def kernel(inputs, W):
    from concourse import bass_utils

    if "nc" not in _CACHE:
        _CACHE["nc"] = _build_program(mmdt16=_MM16)
    nc = _CACHE["nc"]
    in_maps = _prep_inputs(inputs, W)
    res = bass_utils.run_bass_kernel_spmd(nc, in_maps, list(range(NCORES)))
    out = res.results[0]["out"]
    return np.asarray(out, dtype=np.float32).reshape(BS, NC_, OL)
